# revision 5
# baseline (speedup 1.0000x reference)
"""Distributed 2-layer GAT (nn_AlignHead) on 8 TRN2 NeuronCores.

Strategy: shard nodes (dst) contiguously across 8 cores. Per core:
  Phase A: LayerNorm + h@W1_ext dense matmul -> per-node table rows
           [g1 (c-major, 512) | a_src1 (8) | pad] bf16; AllGather tables.
  Phase B: per dst-tile (128 dsts): dma_gather edge src rows (4 SWDGE
           queues), segment-softmax via indicator-matmul machinery:
           P[e,s] built on DVE (is_equal vs pre-expanded iota), Q = P^T via
           TensorE transpose, a_dst broadcast via Q-matmul, p =
           max(exp(z), exp(0.2 z)) (= exp(leaky_relu(z))), weighted
           aggregation + denominator via per-window matmuls into PSUM,
           normalize, ELU -> h2, dense h2@W2_ext -> table2; AllGather.
  Phase C: conv2 edge phase (1 head replicated to 8 pseudo-heads of 32),
           normalize -> final output rows.

Self-contained: hardcodes the problem shapes; compiles on first call.
"""
import sys
import types

import numpy as np
import ml_dtypes

# ---------------------------------------------------------------- constants
NCORE = 8
N = 50000
E = 500000
D = 256
H1, C1 = 8, 64
DH = 512            # H1*C1
NEG = 0.2
EPS = 1e-5
NLOC = 6250         # nodes per core
NPAD = 6272         # 49*128
T = 49              # dst tiles per core
ROWS = NPAD * NCORE  # 50176 global (padded) table rows
LO = 32768          # int16 gather row limit
ELEM1 = 640         # bf16 elems per conv1 table row (1280 B)
ELEM2 = 384         # bf16 elems per conv2 table row (768 B)
NQ = 4              # swdge queues
BF = ml_dtypes.bfloat16

_cache = {}


def _install_ntff_hook():
    if "antenv.axon_hooks" in sys.modules:
        return
    try:
        import antenv
        mod = types.ModuleType("antenv.axon_hooks")
        _h = [None]
        mod.set_axon_ntff_profile_hook = lambda h: _h.__setitem__(0, h)
        mod.get_axon_ntff_profile_hook = lambda: _h[0]
        sys.modules["antenv.axon_hooks"] = mod
        antenv.axon_hooks = mod
        from trn_agent_boot.trn_boot import _ntff_profile_via_ctypes
        mod.set_axon_ntff_profile_hook(
            _ntff_profile_via_ctypes("/opt/axon/libaxon_pjrt.so"))
    except Exception:
        pass


def _prep_edges(edge_index):
    """Partition + window-pad edges. Returns (NW [T,2], Woff [T,2], Wtot,
    per-core idx arrays [128, Wtot*8] int16, slot arrays [128, SWtot] bf16,
    slot col offsets per tile)."""
    src = np.asarray(edge_index[0]).astype(np.int64)
    dst = np.asarray(edge_index[1]).astype(np.int64)
    loops = np.arange(N, dtype=np.int64)
    src = np.concatenate([src, loops])
    dst = np.concatenate([dst, loops])

    core = dst // NLOC
    ldst = dst % NLOC
    tilei = ldst // 128
    slot = ldst % 128
    srow = (src // NLOC) * NPAD + (src % NLOC)
    run = (srow >= LO).astype(np.int64)

    cnt = np.zeros((NCORE, T, 2), np.int64)
    np.add.at(cnt, (core, tilei, run), 1)
    NW = np.maximum(1, np.ceil(cnt.max(axis=0) / 128).astype(np.int64))  # [T,2]

    Woff = np.zeros((T, 2), np.int64)
    w = 0
    for t in range(T):
        Woff[t, 0] = w
        w += NW[t, 0]
        Woff[t, 1] = w
        w += NW[t, 1]
    Wtot = int(w)

    # slot array column offsets: per (t) aligned to even
    SWoff = np.zeros((T, 2), np.int64)
    sw = 0
    for t in range(T):
        SWoff[t, 0] = sw
        sw += NW[t, 0] + (NW[t, 0] & 1)
        SWoff[t, 1] = sw
        sw += NW[t, 1] + (NW[t, 1] & 1)
    SWtot = int(sw)

    order = np.lexsort((ldst, run, tilei, core))
    src_s = srow[order]
    core_s = core[order]
    tile_s = tilei[order]
    run_s = run[order]
    slot_s = slot[order]

    idx_arrs, slot_arrs = [], []
    # per (core,t,r) segment boundaries in the sorted arrays
    seg_key = ((core_s * T + tile_s) * 2 + run_s)
    bounds = np.searchsorted(seg_key, np.arange(NCORE * T * 2 + 1))
    for c in range(NCORE):
        idx16 = np.zeros((16, Wtot * 8), np.int16)
        slots = np.full((128, SWtot), 128.0, np.float32)
        for t in range(T):
            for r in range(2):
                k = (c * T + t) * 2 + r
                a, b = bounds[k], bounds[k + 1]
                n = b - a
                nw = int(NW[t, r])
                assert n <= nw * 128
                rows = src_s[a:b] - r * LO
                sl = slot_s[a:b]
                j = np.arange(n)
                w0 = int(Woff[t, r])
                idx16[j % 16, w0 * 8 + j // 16] = rows.astype(np.int16)
                s0 = int(SWoff[t, r])
                slots[j % 128, s0 + j // 128] = sl
        idx_arrs.append(np.tile(idx16, (8, 1)))
        slot_arrs.append(slots.astype(BF))
    return NW, Woff, SWoff, Wtot, SWtot, idx_arrs, slot_arrs


def _build(NW, Woff, SWoff, Wtot, SWtot, ln_trivial, b1_zero, b2_zero):
    import concourse.bacc as bacc
    import concourse.mybir as mybir
    import concourse.tile as tile

    f32 = mybir.dt.float32
    bf = mybir.dt.bfloat16
    i16 = mybir.dt.int16
    AF = mybir.ActivationFunctionType
    ALU = mybir.AluOpType
    NWmax = int(NW.max())

    nc = bacc.Bacc("TRN2", target_bir_lowering=False, debug=False,
                   num_devices=NCORE, num_swdge_queues=NQ)

    x_in = nc.declare_dram_parameter("x", [NPAD, D], f32, isOutput=False)
    idx_in = nc.declare_dram_parameter("idx", [128, Wtot * 8], i16, isOutput=False)
    sl_in = nc.declare_dram_parameter("slots", [128, SWtot], bf, isOutput=False)
    w1_in = nc.declare_dram_parameter("w1e", [D, DH + 16], bf, isOutput=False)
    w2_in = nc.declare_dram_parameter("w2e", [DH, D + 2], bf, isOutput=False)
    io_in = nc.declare_dram_parameter("iotax", [128, 128 * NWmax], bf, isOutput=False)
    id_in = nc.declare_dram_parameter("ident", [128, 128], bf, isOutput=False)
    lnw_in = lnb_in = b1_in = b2_in = None
    if not ln_trivial:
        lnw_in = nc.declare_dram_parameter("lnw", [128, D], f32, isOutput=False)
        lnb_in = nc.declare_dram_parameter("lnb", [128, D], f32, isOutput=False)
    if not b1_zero:
        b1_in = nc.declare_dram_parameter("b1r", [128, DH], f32, isOutput=False)
    if not b2_zero:
        b2_in = nc.declare_dram_parameter("b2r", [128, D], f32, isOutput=False)
    out_ext = nc.declare_dram_parameter("out", [NPAD, D], f32, isOutput=True)

    tab1_loc = nc.dram_tensor("tab1_loc", [NPAD, ELEM1], bf)
    tab2_loc = nc.dram_tensor("tab2_loc", [NPAD, ELEM2], bf)

    qrot = [0]

    def nextq():
        q = qrot[0]
        qrot[0] = (q + 1) % NQ
        return q

    with tile.TileContext(nc) as tc:
        with (
            tc.tile_pool(name="const", bufs=1) as cpool,
            tc.tile_pool(name="dram", bufs=1, space="DRAM") as dpool,
        ):
            tab1_full = dpool.tile([ROWS, ELEM1], bf, addr_space="Shared")
            tab2_full = dpool.tile([ROWS, ELEM2], bf, addr_space="Shared")

            # ---- constants to SBUF
            w1e = cpool.tile([128, 2, DH + 16], bf)
            nc.sync.dma_start(w1e[:], w1_in[:].rearrange("(k p) f -> p k f", p=128))
            w2e = cpool.tile([128, 4, D + 2], bf)
            nc.sync.dma_start(w2e[:], w2_in[:].rearrange("(k p) f -> p k f", p=128))
            iotax = cpool.tile([128, 128 * NWmax], bf)
            nc.sync.dma_start(iotax[:], io_in[:])
            ident = cpool.tile([128, 128], bf)
            nc.sync.dma_start(ident[:], id_in[:])
            slots_sb = cpool.tile([128, SWtot], bf)
            nc.sync.dma_start(slots_sb[:], sl_in[:])
            idx_sb = cpool.tile([128, Wtot * 8], i16)
            nc.sync.dma_start(idx_sb[:], idx_in[:])
            adst1 = cpool.tile([128, T * 8], bf)
            adst2 = cpool.tile([128, T], bf)
            if not ln_trivial:
                lnw_sb = cpool.tile([128, D], f32)
                nc.sync.dma_start(lnw_sb[:], lnw_in[:])
                lnb_sb = cpool.tile([128, D], f32)
                nc.sync.dma_start(lnb_sb[:], lnb_in[:])
            if not b1_zero:
                b1_sb = cpool.tile([128, DH], f32)
                nc.sync.dma_start(b1_sb[:], b1_in[:])
            if not b2_zero:
                b2_sb = cpool.tile([128, D], f32)
                nc.sync.dma_start(b2_sb[:], b2_in[:])

            iotax3 = iotax[:].rearrange("p (s w) -> p s w", w=NWmax)

            # ================= PHASE A: LN + dense1 + table1 =================
            pha = tc.tile_pool(name="phA", bufs=3)
            iop = pha.__enter__()
            wk_cm = tc.tile_pool(name="wkA", bufs=2)
            wkp = wk_cm.__enter__()
            sm_cm = tc.tile_pool(name="smA", bufs=3)
            smp = sm_cm.__enter__()
            psA_cm = tc.tile_pool(name="psA", bufs=2, space="PSUM")
            psA = psA_cm.__enter__()
            psT_cm = tc.tile_pool(name="psTA", bufs=2, space="PSUM")
            psT = psT_cm.__enter__()
            for t in range(T):
                xt = iop.tile([128, D], f32, tag="xt")
                nc.sync.dma_start(xt[:], x_in[t * 128:(t + 1) * 128, :])
                mean = smp.tile([128, 1], f32, tag="mean")
                nc.vector.reduce_sum(mean[:], xt[:], axis=mybir.AxisListType.X)
                nc.vector.tensor_scalar_mul(mean[:], mean[:], 1.0 / D)
                xc = wkp.tile([128, D], f32, tag="xc")
                nc.vector.tensor_scalar(xc[:], xt[:], mean[:], None, ALU.subtract)
                sq = smp.tile([128, 1], f32, tag="sq")
                sqj = wkp.tile([128, D], f32, tag="sqj")
                nc.scalar.activation(sqj[:], xc[:], AF.Square, accum_out=sq[:])
                nc.vector.tensor_scalar(sq[:], sq[:], 1.0 / D, EPS, ALU.mult, ALU.add)
                sd = smp.tile([128, 1], f32, tag="sd")
                nc.scalar.activation(sd[:], sq[:], AF.Sqrt)
                rstd = smp.tile([128, 1], f32, tag="rstd")
                nc.vector.reciprocal(rstd[:], sd[:])
                hbf = wkp.tile([128, D], bf, tag="hbf")
                if ln_trivial:
                    nc.scalar.activation(hbf[:], xc[:], AF.Copy, scale=rstd[:])
                else:
                    hf = wkp.tile([128, D], f32, tag="hf")
                    nc.scalar.activation(hf[:], xc[:], AF.Copy, scale=rstd[:])
                    nc.vector.tensor_mul(hf[:], hf[:], lnw_sb[:])
                    nc.vector.tensor_add(hbf[:], hf[:], lnb_sb[:])
                # transpose h -> [feat, node]
                hT = wkp.tile([128, 2, 128], bf, tag="hT")
                for k in range(2):
                    pst = psT.tile([128, 128], bf, tag="pstA")
                    nc.tensor.transpose(pst[:], hbf[:, k * 128:(k + 1) * 128], ident[:])
                    nc.scalar.copy(hT[:, k, :], pst[:])
                ps1 = psA.tile([128, DH], f32, tag="ps1")
                ps1b = psA.tile([128, 16], f32, tag="ps1b")
                for k in range(2):
                    nc.tensor.matmul(ps1[:], hT[:, k, :], w1e[:, k, 0:DH],
                                     start=(k == 0), stop=(k == 1))
                    nc.tensor.matmul(ps1b[:], hT[:, k, :],
                                     w1e[:, k, DH:DH + 16],
                                     start=(k == 0), stop=(k == 1))
                nc.scalar.copy(adst1[:, t * 8:(t + 1) * 8], ps1b[:, 8:16])
                tb = iop.tile([128, ELEM1], bf, tag="tb1")
                # ps1 is already c-major (W1e columns pre-permuted on host)
                nc.scalar.copy(tb[:, 0:DH], ps1[:])
                nc.scalar.copy(tb[:, DH:DH + 8], ps1b[:, 0:8])
                nc.sync.dma_start(tab1_loc[t * 128:(t + 1) * 128, 0:DH + 8], tb[:, 0:DH + 8])

            psT_cm.__exit__(None, None, None)
            psA_cm.__exit__(None, None, None)
            sm_cm.__exit__(None, None, None)
            wk_cm.__exit__(None, None, None)
            pha.__exit__(None, None, None)

            nc.gpsimd.collective_compute(
                "AllGather", mybir.AluOpType.bypass,
                replica_groups=[list(range(NCORE))],
                ins=[tab1_loc[:]], outs=[tab1_full.opt()],
            )

            # ================= PHASE B: conv1 edges + dense2 =================
            NWT1 = int((NW[:, 0] + NW[:, 1]).max())
            phb = tc.tile_pool(name="phB", bufs=3)
            iop = phb.__enter__()
            wk_cm = tc.tile_pool(name="wkB", bufs=2)
            wkp = wk_cm.__enter__()
            sm_cm = tc.tile_pool(name="smB", bufs=3)
            smp = sm_cm.__enter__()
            ga_cm = tc.tile_pool(name="gaB", bufs=2)
            gap = ga_cm.__enter__()
            st_cm = tc.tile_pool(name="stB", bufs=2)
            stp = st_cm.__enter__()
            psQ_cm = tc.tile_pool(name="psQ", bufs=1, space="PSUM")
            psQ = psQ_cm.__enter__()
            psZ_cm = tc.tile_pool(name="psZ", bufs=1, space="PSUM")
            psZ = psZ_cm.__enter__()
            psC_cm = tc.tile_pool(name="psC", bufs=1, space="PSUM")
            psC = psC_cm.__enter__()
            psT_cm = tc.tile_pool(name="psTB", bufs=2, space="PSUM")
            psT = psT_cm.__enter__()
            for t in range(T):
                nwl, nwh = int(NW[t, 0]), int(NW[t, 1])
                nwt = nwl + nwh
                gt = gap.tile([128, NWT1, ELEM1], bf, tag="gt1")
                for (rbase, w0g, w0l, nw) in _calls(t, nwl, nwh, Woff):
                    src_ap = tab1_full[0:LO, :] if rbase == 0 else tab1_full[LO:ROWS, :]
                    nc.gpsimd.dma_gather(
                        gt[:, w0l:w0l + nw, :], src_ap,
                        idx_sb[:, w0g * 8:(w0g + nw) * 8],
                        num_idxs=nw * 128, num_idxs_reg=nw * 128,
                        elem_size=ELEM1, queue_num=nextq(),
                    )
                P = stp.tile([128, 128 * NWT1], bf, tag="P1")
                Pv = P[:, :128 * nwt].rearrange("p (s w) -> p s w", w=nwt)
                sl_lo = slots_sb[:, int(SWoff[t, 0]):int(SWoff[t, 0]) + nwl]
                sl_hi = slots_sb[:, int(SWoff[t, 1]):int(SWoff[t, 1]) + nwh]
                nc.vector.tensor_tensor(
                    Pv[:, :, 0:nwl],
                    sl_lo.unsqueeze(1).broadcast_to([128, 128, nwl]),
                    iotax3[:, :, 0:nwl], ALU.is_equal)
                nc.vector.tensor_tensor(
                    Pv[:, :, nwl:nwt],
                    sl_hi.unsqueeze(1).broadcast_to([128, 128, nwh]),
                    iotax3[:, :, 0:nwh], ALU.is_equal)
                Qp = psQ.tile([128, NWT1 * 128], bf, tag="Qp1")
                for w in range(nwt):
                    nc.tensor.transpose(Qp[:, w * 128:(w + 1) * 128],
                                        Pv[:, :, w], ident[:])
                Q = stp.tile([128, NWT1 * 128], bf, tag="Q1")
                nc.scalar.copy(Q[:, :nwt * 128], Qp[:, :nwt * 128])
                zb = psZ.tile([128, NWT1 * 8], f32, tag="zb1")
                for w in range(nwt):
                    nc.tensor.matmul(zb[:, w * 8:(w + 1) * 8],
                                     Q[:, w * 128:(w + 1) * 128],
                                     adst1[:, t * 8:(t + 1) * 8],
                                     start=True, stop=True)
                z = smp.tile([128, NWT1 * 8], f32, tag="z1")
                nc.vector.scalar_tensor_tensor(
                    z[:, :nwt * 8].rearrange("p (w d) -> p w d", d=8),
                    zb[:, :nwt * 8].rearrange("p (w d) -> p w d", d=8), 1.0,
                    gt[:, 0:nwt, DH:DH + 8],
                    ALU.mult, ALU.add)
                e2 = smp.tile([128, NWT1 * 8], f32, tag="e21")
                nc.scalar.activation(e2[:, :nwt * 8], z[:, :nwt * 8], AF.Exp, scale=NEG)
                e1 = smp.tile([128, NWT1 * 8], f32, tag="e11")
                nc.scalar.activation(e1[:, :nwt * 8], z[:, :nwt * 8], AF.Exp)
                stg = stp.tile([128, NWT1, 8 + DH], bf, tag="stg1")
                nc.vector.tensor_tensor(
                    stg[:, 0:nwt, 0:8],
                    e1[:, :nwt * 8].rearrange("p (w d) -> p w d", d=8),
                    e2[:, :nwt * 8].rearrange("p (w d) -> p w d", d=8),
                    ALU.max)
                # W'' = g (c-major) * p-bcast
                nc.vector.tensor_mul(
                    stg[:, 0:nwt, 8:8 + DH].rearrange("p w (c h) -> p w c h", h=8),
                    gt[:, 0:nwt, 0:DH].rearrange("p w (c h) -> p w c h", h=8),
                    stg[:, 0:nwt, 0:8].unsqueeze(2).broadcast_to([128, nwt, 64, 8]))
                ocd = psZ.tile([128, 8], f32, tag="ocd1")
                oco = psC.tile([128, DH], f32, tag="oco1")
                for w in range(nwt):
                    nc.tensor.matmul(ocd[:], Pv[:, :, w], stg[:, w, 0:8],
                                     start=(w == 0), stop=(w == nwt - 1))
                    nc.tensor.matmul(oco[:], Pv[:, :, w], stg[:, w, 8:8 + DH],
                                     start=(w == 0), stop=(w == nwt - 1))
                den = smp.tile([128, 8], f32, tag="den1")
                nc.vector.tensor_scalar_max(den[:], ocd[:], 1e-30)
                rec = smp.tile([128, 8], f32, tag="rec1")
                nc.vector.reciprocal(rec[:], den[:])
                o1 = wkp.tile([128, DH], bf, tag="o1")
                nc.vector.tensor_tensor(
                    o1[:].rearrange("p (c h) -> p c h", h=8),
                    oco[:].rearrange("p (c h) -> p c h", h=8),
                    rec[:].unsqueeze(1).broadcast_to([128, 64, 8]),
                    ALU.mult)
                if not b1_zero:
                    o1f = wkp.tile([128, DH], f32, tag="o1f")
                    nc.vector.tensor_add(o1f[:], o1[:], b1_sb[:])
                    o1 = o1f
                # ELU: h2 = relu(u) + exp(-relu(-u)) - 1
                pos = wkp.tile([128, DH], bf, tag="pos")
                nc.scalar.activation(pos[:], o1[:], AF.Relu)
                rneg = wkp.tile([128, DH], bf, tag="rneg")
                nc.scalar.activation(rneg[:], o1[:], AF.Relu, scale=-1.0)
                en = wkp.tile([128, DH], bf, tag="en")
                nc.scalar.activation(en[:], rneg[:], AF.Exp, scale=-1.0)
                h2 = wkp.tile([128, DH], bf, tag="h2")
                nc.vector.scalar_tensor_tensor(h2[:], pos[:], -1.0, en[:],
                                               ALU.add, ALU.add)
                # dense2
                hT2 = wkp.tile([128, 4, 128], bf, tag="hT2")
                for k in range(4):
                    pst = psT.tile([128, 128], bf, tag="pstB")
                    nc.tensor.transpose(pst[:], h2[:, k * 128:(k + 1) * 128], ident[:])
                    nc.scalar.copy(hT2[:, k, :], pst[:])
                ps2 = psQ.tile([128, D + 2], f32, tag="Qp1")
                for k in range(4):
                    nc.tensor.matmul(ps2[:], hT2[:, k, :], w2e[:, k, :],
                                     start=(k == 0), stop=(k == 3))
                nc.scalar.copy(adst2[:, t:t + 1], ps2[:, D + 1:D + 2])
                tb2 = iop.tile([128, ELEM2], bf, tag="tb2")
                nc.scalar.copy(tb2[:, 0:D + 1], ps2[:, 0:D + 1])
                nc.sync.dma_start(tab2_loc[t * 128:(t + 1) * 128, 0:D + 1], tb2[:, 0:D + 1])

            psT_cm.__exit__(None, None, None)
            psC_cm.__exit__(None, None, None)
            psZ_cm.__exit__(None, None, None)
            psQ_cm.__exit__(None, None, None)
            st_cm.__exit__(None, None, None)
            ga_cm.__exit__(None, None, None)
            sm_cm.__exit__(None, None, None)
            wk_cm.__exit__(None, None, None)
            phb.__exit__(None, None, None)

            nc.gpsimd.collective_compute(
                "AllGather", mybir.AluOpType.bypass,
                replica_groups=[list(range(NCORE))],
                ins=[tab2_loc[:]], outs=[tab2_full.opt()],
            )

            # ================= PHASE C: conv2 edges =================
            phc = tc.tile_pool(name="phC", bufs=3)
            iop = phc.__enter__()
            sm_cm = tc.tile_pool(name="smC", bufs=3)
            smp = sm_cm.__enter__()
            ga_cm = tc.tile_pool(name="gaC", bufs=2)
            gap = ga_cm.__enter__()
            st_cm = tc.tile_pool(name="stC", bufs=2)
            stp = st_cm.__enter__()
            psQ_cm = tc.tile_pool(name="psQC", bufs=1, space="PSUM")
            psQ = psQ_cm.__enter__()
            psZ_cm = tc.tile_pool(name="psZC", bufs=1, space="PSUM")
            psZ = psZ_cm.__enter__()
            psC_cm = tc.tile_pool(name="psCC", bufs=2, space="PSUM")
            psC = psC_cm.__enter__()
            for t in range(T):
                nwl, nwh = int(NW[t, 0]), int(NW[t, 1])
                nwt = nwl + nwh
                gt = gap.tile([128, NWT1, ELEM2], bf, tag="gt2")
                for (rbase, w0g, w0l, nw) in _calls(t, nwl, nwh, Woff):
                    src_ap = tab2_full[0:LO, :] if rbase == 0 else tab2_full[LO:ROWS, :]
                    nc.gpsimd.dma_gather(
                        gt[:, w0l:w0l + nw, :], src_ap,
                        idx_sb[:, w0g * 8:(w0g + nw) * 8],
                        num_idxs=nw * 128, num_idxs_reg=nw * 128,
                        elem_size=ELEM2, queue_num=nextq(),
                    )
                P = stp.tile([128, 128 * NWT1], bf, tag="P1")
                Pv = P[:, :128 * nwt].rearrange("p (s w) -> p s w", w=nwt)
                sl_lo = slots_sb[:, int(SWoff[t, 0]):int(SWoff[t, 0]) + nwl]
                sl_hi = slots_sb[:, int(SWoff[t, 1]):int(SWoff[t, 1]) + nwh]
                nc.vector.tensor_tensor(
                    Pv[:, :, 0:nwl],
                    sl_lo.unsqueeze(1).broadcast_to([128, 128, nwl]),
                    iotax3[:, :, 0:nwl], ALU.is_equal)
                nc.vector.tensor_tensor(
                    Pv[:, :, nwl:nwt],
                    sl_hi.unsqueeze(1).broadcast_to([128, 128, nwh]),
                    iotax3[:, :, 0:nwh], ALU.is_equal)
                Qp = psQ.tile([128, NWT1 * 128], bf, tag="Qp2")
                for w in range(nwt):
                    nc.tensor.transpose(Qp[:, w * 128:(w + 1) * 128],
                                        Pv[:, :, w], ident[:])
                Q = stp.tile([128, NWT1 * 128], bf, tag="Q2")
                nc.scalar.copy(Q[:, :nwt * 128], Qp[:, :nwt * 128])
                zb = psZ.tile([128, NWT1], f32, tag="zb2")
                for w in range(nwt):
                    nc.tensor.matmul(zb[:, w:w + 1],
                                     Q[:, w * 128:(w + 1) * 128],
                                     adst2[:, t:t + 1],
                                     start=True, stop=True)
                z = smp.tile([128, NWT1], f32, tag="z2")
                nc.vector.scalar_tensor_tensor(
                    z[:, :nwt].rearrange("p (w d) -> p w d", d=1),
                    zb[:, :nwt].rearrange("p (w d) -> p w d", d=1), 1.0,
                    gt[:, 0:nwt, D:D + 1],
                    ALU.mult, ALU.add)
                e2 = smp.tile([128, NWT1], f32, tag="e22")
                nc.scalar.activation(e2[:, :nwt], z[:, :nwt], AF.Exp, scale=NEG)
                e1 = smp.tile([128, NWT1], f32, tag="e12")
                nc.scalar.activation(e1[:, :nwt], z[:, :nwt], AF.Exp)
                p2 = smp.tile([128, NWT1], f32, tag="p2")
                nc.vector.tensor_max(p2[:, :nwt], e1[:, :nwt], e2[:, :nwt])
                stg = stp.tile([128, NWT1, 8 + D], bf, tag="stg2")
                # replicate p2 into 8 pseudo-head cols
                nc.scalar.copy(
                    stg[:, 0:nwt, 0:8],
                    p2[:, :nwt].unsqueeze(2).broadcast_to([128, nwt, 8]))
                nc.vector.tensor_mul(
                    stg[:, 0:nwt, 8:8 + D].rearrange("p w (c h) -> p w c h", h=8),
                    gt[:, 0:nwt, 0:D].rearrange("p w (c h) -> p w c h", h=8),
                    stg[:, 0:nwt, 0:8].unsqueeze(2).broadcast_to([128, nwt, 32, 8]))
                ocd = psZ.tile([128, 8], f32, tag="ocd2")
                oco = psC.tile([128, D], f32, tag="oco2")
                for w in range(nwt):
                    nc.tensor.matmul(ocd[:], Pv[:, :, w], stg[:, w, 0:8],
                                     start=(w == 0), stop=(w == nwt - 1))
                    nc.tensor.matmul(oco[:], Pv[:, :, w], stg[:, w, 8:8 + D],
                                     start=(w == 0), stop=(w == nwt - 1))
                den = smp.tile([128, 1], f32, tag="den2")
                nc.vector.tensor_scalar_max(den[:], ocd[:, 0:1], 1e-30)
                rec = smp.tile([128, 1], f32, tag="rec2")
                nc.vector.reciprocal(rec[:], den[:])
                outt = iop.tile([128, D], f32, tag="outt")
                # un-permute c-major -> natural: out[h*32+c] = oc[8 + c*8+h]
                nc.vector.tensor_scalar(
                    outt[:].rearrange("p (h c) -> p h c", c=32),
                    oco[:].rearrange("p (c h) -> p c h", h=8).transpose([0, 2, 1]),
                    rec[:], None, ALU.mult)
                if not b2_zero:
                    nc.vector.tensor_add(outt[:], outt[:], b2_sb[:])
                nc.sync.dma_start(out_ext[t * 128:(t + 1) * 128, :], outt[:])
            psC_cm.__exit__(None, None, None)
            psZ_cm.__exit__(None, None, None)
            psQ_cm.__exit__(None, None, None)
            st_cm.__exit__(None, None, None)
            ga_cm.__exit__(None, None, None)
            sm_cm.__exit__(None, None, None)
            phc.__exit__(None, None, None)

    nc.compile()
    return nc


def _calls(t, nwl, nwh, Woff):
    """Gather call plan for tile t: (run_base, global_w0, local_w0, nw)."""
    out = []
    for r, nwr, lbase in ((0, nwl, 0), (1, nwh, nwl)):
        w0 = int(Woff[t, r])
        done = 0
        while done < nwr:
            nw = min(8, nwr - done)
            out.append((r, w0 + done, lbase + done, nw))
            done += nw
    return out


def _host_prep(inputs):
    edge_index = np.asarray(inputs["edge_index"])
    x = np.asarray(inputs["x"], np.float32)
    ln_w = np.asarray(inputs["ln_w"], np.float32)
    ln_b = np.asarray(inputs["ln_b"], np.float32)
    W1 = np.asarray(inputs["W1"], np.float32)
    a_s1 = np.asarray(inputs["att_src1"], np.float32)
    a_d1 = np.asarray(inputs["att_dst1"], np.float32)
    b1 = np.asarray(inputs["b1"], np.float32)
    W2 = np.asarray(inputs["W2"], np.float32)
    a_s2 = np.asarray(inputs["att_src2"], np.float32)
    a_d2 = np.asarray(inputs["att_dst2"], np.float32)
    b2 = np.asarray(inputs["b2"], np.float32)

    NW, Woff, SWoff, Wtot, SWtot, idx_arrs, slot_arrs = _prep_edges(edge_index)
    NWmax = int(NW.max())

    # W1_ext: c-major permuted cols + attention folds
    perm1 = np.empty(DH, np.int64)
    for h in range(H1):
        for c in range(C1):
            perm1[c * 8 + h] = h * C1 + c
    W1p = W1[:, perm1]
    wsrc1 = np.stack([W1[:, h * C1:(h + 1) * C1] @ a_s1[h] for h in range(H1)], 1)
    wdst1 = np.stack([W1[:, h * C1:(h + 1) * C1] @ a_d1[h] for h in range(H1)], 1)
    w1e = np.concatenate([W1p, wsrc1, wdst1], axis=1).astype(BF)  # [256, 528]

    # W2_ext: rows permuted to h2's c-major layout; cols permuted to
    # pseudo-head c-major (8 groups of 32); + attention folds
    W2r = W2[perm1, :]
    perm2 = np.empty(D, np.int64)
    for h in range(8):
        for c in range(32):
            perm2[c * 8 + h] = h * 32 + c
    W2p = W2r[:, perm2]
    wsrc2 = W2r @ a_s2[0]
    wdst2 = W2r @ a_d2[0]
    w2e = np.concatenate([W2p, wsrc2[:, None], wdst2[:, None]], axis=1).astype(BF)

    iotax = np.zeros((128, 128 * NWmax), np.float32)
    for s in range(128):
        iotax[:, s * NWmax:(s + 1) * NWmax] = s
    iotax = iotax.astype(BF)
    identm = np.eye(128).astype(BF)

    ln_trivial = bool(np.all(ln_w == 1.0) and np.all(ln_b == 0.0))
    b1_zero = bool(np.all(b1 == 0.0))
    b2_zero = bool(np.all(b2 == 0.0))

    in_maps = []
    for c in range(NCORE):
        xp = np.zeros((NPAD, D), np.float32)
        xp[:NLOC] = x[c * NLOC:(c + 1) * NLOC]
        m = {
            "x": xp, "idx": idx_arrs[c], "slots": slot_arrs[c],
            "w1e": w1e, "w2e": w2e, "iotax": iotax, "ident": identm,
        }
        if not ln_trivial:
            m["lnw"] = np.tile(ln_w[None, :], (128, 1)).astype(np.float32)
            m["lnb"] = np.tile(ln_b[None, :], (128, 1)).astype(np.float32)
        if not b1_zero:
            m["b1r"] = np.tile(b1[perm1][None, :], (128, 1)).astype(np.float32)
        if not b2_zero:
            m["b2r"] = np.tile(b2[None, :], (128, 1)).astype(np.float32)
        in_maps.append(m)
    meta = (NW, Woff, SWoff, Wtot, SWtot, ln_trivial, b1_zero, b2_zero)
    return meta, in_maps


def kernel(**inputs):
    _install_ntff_hook()
    from concourse.bass_utils import run_bass_kernel_spmd

    meta, in_maps = _host_prep(inputs)
    NW, Woff, SWoff, Wtot, SWtot, ln_trivial, b1_zero, b2_zero = meta
    key = (Wtot, SWtot, ln_trivial, b1_zero, b2_zero, NW.tobytes())
    if key not in _cache:
        _cache[key] = _build(NW, Woff, SWoff, Wtot, SWtot,
                             ln_trivial, b1_zero, b2_zero)
    nc = _cache[key]

    trace = bool(int(__import__("os").environ.get("KERNEL_TRACE", "0")))
    res = run_bass_kernel_spmd(nc, in_maps, core_ids=list(range(NCORE)),
                               trace=trace)
    kernel.last_exec_time_ns = res.exec_time_ns
    out = np.concatenate([res.results[c]["out"][:NLOC] for c in range(NCORE)], 0)
    return out.astype(np.float32)


kernel.last_exec_time_ns = None
